# revision 26
# baseline (speedup 1.0000x reference)
"""Single-head causal attention kernel for Trainium2 (Bass/Tile), SPMD over 8 cores.

Problem: inputs [B=8, S=2048, E=1024]; Wq/Wk/Wv [E, H=1024]; bq/bk/bv [H].
  q = x@Wq+bq; k = x@Wk+bk; v = x@Wv+bv
  out = softmax(causal(q k^T / sqrt(H))) v        -> [B, S, H]

Sharding: data-parallel over batch, 1 batch element per NeuronCore (8 cores).

Strategy (v4, bf16 + merged-QK):  softmax is invariant to per-row shifts, so
  q k^T = (x Wq + bq)(x Wk + bk)^T
        = x (Wq Wk^T) x^T + [per-key bias beta_j = x_j.(Wk bq)] + row-consts.
Host precomputes M = Wq Wk^T (f32) and beta (f32), so the device needs only
TWO projections (G^T = M^T x^T and V) instead of three; beta rides for free as
the per-partition bias of the exp() eviction.  x^T stays resident in SBUF and
serves as the stationary side of the scores matmuls (the role K^T played).

  phase 1 (per 512-wide s-chunk): G^T[f,s] (stationary M^T tiles, plain
           eviction), then V[s,h] (stationary x^T tiles; bias added during
           eviction from a host-broadcast [128,H] bv tile). All matmuls N=512.
  phase 2 (q-chunks processed in reverse so the tail chain is the smallest):
           scoresT[k,q] matmuls (stationary x^T k-slices, moving G^T),
           exp(x/32 + beta_k/32) on ScalarE, causal edge mask via
           gpsimd.affine_select; AV + Z share one stationary load per attnT
           tile (fully-masked diagonal tiles skipped for even q-subtiles);
           1/Z folded into the PSUM eviction. AV of a chunk is issued after
           the next chunk's scores so the PE in-order queue never stalls on
           ScalarE.
  A short burst of dummy matmuls runs during the initial DMA wait to lift
  the PE HAM clock-gate to 8/8 before real work starts.
"""

import numpy as np
import ml_dtypes

import concourse.bacc as bacc
import concourse.mybir as mybir
from concourse import tile
from concourse import bass_utils

P = 128
F32 = mybir.dt.float32
BF16 = mybir.dt.bfloat16

B, S, E, H = 8, 2048, 1024, 1024
QC = 256          # q-chunk width in attention phase
N_CORES = 8
NPBF16 = ml_dtypes.bfloat16


def attention_kernel(tc, out, xt, mtp, wv, bvb, bsc, vones):
    nc = tc.nc
    ST, ET, HT = S // P, E // P, H // P     # 128-tiles per dim
    NSC = S // 512                          # 512-wide s-chunks
    NQC = S // QC                           # q-chunks
    QSUB = QC // P
    inv_sqrt_h = 1.0 / float(np.sqrt(H))

    from contextlib import ExitStack

    root = ExitStack()
    with root:
        # ---- constants ----
        const = root.enter_context(tc.tile_pool(name="const", bufs=1))
        warm_src = const.tile([P, 512], BF16, name="warm_src")
        nc.gpsimd.memset(warm_src, 0.0)
        bsc_sb = const.tile([P, ST], F32, name="bsc_sb")
        bv_sb = const.tile([P, H], BF16, name="bv_sb")

        # ---- resident arrays ----
        kqv_pool = root.enter_context(tc.tile_pool(name="kqv", bufs=1))
        # chunk-major: [:, c, e, :] is one contiguous 8KB/partition DMA
        xt_sb = kqv_pool.tile([P, NSC, ET, 512], BF16, name="xt_sb")
        gt = kqv_pool.tile([P, HT, S], BF16, name="gt")      # G^T [f,s]
        # V [s,h] padded with two ones-columns at h=H,H+1 so the softmax
        # row-sum Z rides the AV matmuls (no separate N=1 Z matmuls) and the
        # three AV slices are a uniform 342 wide.
        HP = H + 2
        v_sb = kqv_pool.tile([P, ST, HP], BF16, name="v_sb")

        # ================= phase 1: projections =================
        with ExitStack() as ph1:
            w_pool = ph1.enter_context(tc.tile_pool(name="w", bufs=1))
            # mg_sb[:, t, e, :] = M[e*128+p, t*128+c]  (host pre-tiled)
            mg_sb = w_pool.tile([P, HT, ET, P], BF16, name="mg_sb")
            wv_sb = w_pool.tile([P, ET, H], BF16, name="wv_sb")

            # DMA order. Engines boot staggered: Scalar's queue is live at
            # ~3.5us while Sync (booted last) can't trigger before ~10us, so
            # the critical head of the stream (M tiles 0-2 + xt chunk 0)
            # goes out on the Scalar queue; the rest follows on Sync (each
            # trigger costs ~0.6us of issue time, so few, large transfers).
            CW = ET * 512                        # xt cols per chunk
            nc.scalar.dma_start(mg_sb[:, 0, :, :], mtp[0:P, :])
            nc.scalar.dma_start(xt_sb[:, 0, :, :], xt[:, 0:CW])
            nc.scalar.dma_start(mg_sb[:, 1, :, :], mtp[P:2 * P, :])
            nc.scalar.dma_start(mg_sb[:, 2, :, :], mtp[2 * P:3 * P, :])
            for t in range(3, HT):
                nc.sync.dma_start(mg_sb[:, t, :, :], mtp[t * P:(t + 1) * P, :])
            for c in range(1, NSC):
                nc.sync.dma_start(xt_sb[:, c, :, :],
                                  xt[:, c * CW:(c + 1) * CW])
            for e in range(ET):
                nc.sync.dma_start(wv_sb[:, e, :], wv[e * P:(e + 1) * P, :])
            nc.sync.dma_start(bv_sb[:], bvb)
            nc.sync.dma_start(bsc_sb[:], bsc)
            nc.sync.dma_start(v_sb[:, :, H:HP], vones)

            # gpsum first: phase-2 pools then reuse banks that phase 1
            # frees early (after G) rather than last (after V's evictions).
            gpsum = ph1.enter_context(tc.tile_pool(name="gpsum", bufs=2,
                                                   space="PSUM"))
            vpsum = ph1.enter_context(tc.tile_pool(name="vpsum", bufs=2,
                                                   space="PSUM"))
            # HAM warmup: dummy matmuls with no DMA dependency fill the
            # initial DMA wait and lift the PE clock gate to 8/8.
            wp = gpsum.tile([P, 512], F32, name="g0", space="PSUM")
            for _ in range(3):
                nc.tensor.matmul(wp[:], warm_src[:, 0:P], warm_src[:],
                                 start=True, stop=True)

            def evict_g(t, c, psum, alt):
                if alt % 2 == 0:
                    nc.scalar.copy(gt[:, t, c * 512:(c + 1) * 512], psum[:])
                else:
                    nc.vector.tensor_copy(gt[:, t, c * 512:(c + 1) * 512],
                                          psum[:])

            def v_chunk(c):
                for si in range(4 * c, 4 * c + 4):
                    vps = []
                    for hc in range(2):
                        vp = vpsum.tile([P, 512], F32, name="vp", space="PSUM")
                        for e in range(ET):
                            nc.tensor.matmul(
                                vp[:],
                                xt_sb[:, c, e, (si % 4) * P:(si % 4 + 1) * P],
                                wv_sb[:, e, hc * 512:(hc + 1) * 512],
                                start=(e == 0), stop=(e == ET - 1))
                        vps.append(vp)
                    for hc in range(2):
                        nc.vector.scalar_tensor_tensor(
                            v_sb[:, si, hc * 512:(hc + 1) * 512], vps[hc][:],
                            1.0, bv_sb[:, hc * 512:(hc + 1) * 512],
                            mybir.AluOpType.mult, mybir.AluOpType.add)

            # ---- G: plain per-chunk loops; chunk-local deps keep the PE
            # fed as each xt chunk lands (chunk c is consumed over ~13.6us
            # while chunk c+1 streams in).
            for c in range(NSC):
                for t in range(HT):
                    gp = gpsum.tile([P, 512], F32, name=f"g{c % 2}",
                                    space="PSUM")
                    for e in range(ET):
                        nc.tensor.matmul(
                            gp[:], mg_sb[:, t, e, :], xt_sb[:, c, e, :],
                            start=(e == 0), stop=(e == ET - 1))
                    evict_g(t, c, gp, t + c)
            for c in range(NSC):
                v_chunk(c)

        # ================= phase 2: attention =================
        with ExitStack() as ph2:
            attn_pool = ph2.enter_context(
                tc.tile_pool(name="attnT", bufs=36))
            o_pool = ph2.enter_context(tc.tile_pool(name="o_stage", bufs=3))
            rz_pool = ph2.enter_context(tc.tile_pool(name="rz", bufs=4))
            spsum = ph2.enter_context(tc.tile_pool(name="spsum", bufs=2,
                                                   space="PSUM"))
            opsum = ph2.enter_context(tc.tile_pool(name="opsum", bufs=6,
                                                   space="PSUM"))

            def scores_chunk(j):
                """ScoresT tiles [k,q] + exp + causal mask for q-chunk j."""
                nk = ((j + 1) * QC) // P
                ats = []
                for i in range(nk):
                    # The last k-tile (i == 2j+1) lies above the diagonal for
                    # the first q-subtile; only its [:, P:2P] half is ever
                    # read by AV/Z, so compute just those 128 columns.
                    lo = P if i == nk - 1 else 0
                    sp = spsum.tile([P, QC], F32, name="sp", space="PSUM")
                    for t in range(HT):
                        nc.tensor.matmul(
                            sp[:, 0:QC - lo],
                            xt_sb[:, i // 4, t,
                                  (i % 4) * P:(i % 4 + 1) * P],
                            gt[:, t, j * QC + lo:(j + 1) * QC],
                            start=(t == 0), stop=(t == HT - 1))
                    at = attn_pool.tile([P, QC], BF16, name="at")
                    nc.scalar.activation(at[:, lo:QC], sp[:, 0:QC - lo],
                                         mybir.ActivationFunctionType.Exp,
                                         scale=inv_sqrt_h,
                                         bias=bsc_sb[:, i:i + 1])
                    if (i + 1) * P > j * QC + lo:   # tile touches the diagonal
                        nc.gpsimd.affine_select(
                            out=at[:, lo:QC], in_=at[:, lo:QC],
                            compare_op=mybir.AluOpType.is_ge,
                            fill=0.0,
                            base=j * QC + lo - i * P,
                            channel_multiplier=-1,
                            pattern=[[1, QC - lo]])
                    ats.append(at)
                return ats

            # AV splits H+2 (V plus the ones columns) into 3 PSUM-bank-sized
            # matmuls; Z comes out as the last columns of the third slice.
            AVS = [(0, 342), (342, 342), (684, 342)]

            def av_chunk(j, ats):
                """AV + Z for q-chunk j given its masked attnT tiles."""
                for qs in range(QSUB - 1, -1, -1):   # larger qs first
                    # causal: k-tiles above the diagonal for this q-subtile
                    # are fully masked; skip them.
                    nk = 2 * j + qs + 1
                    ops = [opsum.tile([P, 342], F32, name="op", space="PSUM")
                           for _ in range(3)]
                    for i in range(nk):
                        a_sl = ats[i][:, qs * P:(qs + 1) * P]
                        for sl, (off, w) in enumerate(AVS):
                            nc.tensor.matmul(
                                ops[sl][:, 0:w], a_sl,
                                v_sb[:, i, off:off + w],
                                start=(i == 0), stop=(i == nk - 1))
                    rz = rz_pool.tile([P, 1], F32, name="rz")
                    nc.vector.reciprocal(rz[:], ops[2][:, 340:341])
                    o_st = o_pool.tile([P, H], BF16, name="o_st")
                    row = j * QC + qs * P
                    # eviction muls split across Vector and Scalar so the
                    # per-subtile serial chain stays under the AV matmul time
                    nc.vector.tensor_scalar_mul(
                        o_st[:, 0:342], ops[0][:, 0:342], rz[:, 0:1])
                    nc.scalar.mul(o_st[:, 342:684], ops[1][:, 0:342],
                                  rz[:, 0:1])
                    if qs % 2 == 0:
                        nc.vector.tensor_scalar_mul(
                            o_st[:, 684:1024], ops[2][:, 0:340], rz[:, 0:1])
                    else:
                        nc.scalar.mul(o_st[:, 684:1024], ops[2][:, 0:340],
                                      rz[:, 0:1])
                    nc.sync.dma_start(out[row:row + P, :], o_st[:])

            prev = None
            prev_j = None
            for j in range(NQC - 1, -1, -1):     # reverse: smallest AV last
                ats = scores_chunk(j)
                if prev is not None:
                    av_chunk(prev_j, prev)
                prev, prev_j = ats, j
            av_chunk(prev_j, prev)


def build_program(n_cores=N_CORES):
    nc = bacc.Bacc("TRN2", target_bir_lowering=False, debug=False,
                   num_devices=n_cores)
    xt = nc.dram_tensor("xt", [P, S * E // P], BF16, kind="ExternalInput").ap()
    mtp = nc.dram_tensor("mtp", [H, E], BF16, kind="ExternalInput").ap()
    wv = nc.dram_tensor("wv", [E, H], BF16, kind="ExternalInput").ap()
    bvb = nc.dram_tensor("bvb", [P, H], BF16, kind="ExternalInput").ap()
    bsc = nc.dram_tensor("bsc", [P, S // P], F32, kind="ExternalInput").ap()
    vones = nc.dram_tensor("vones", [P, S // P, 2], BF16,
                           kind="ExternalInput").ap()
    out = nc.dram_tensor("out", [S, H], BF16, kind="ExternalOutput").ap()
    with tile.TileContext(nc) as tc:
        attention_kernel(tc, out, xt, mtp, wv, bvb, bsc, vones)
    nc.compile()
    return nc


def _tile_by_h(w):
    """[E,H] -> [H,E] layout where row t*128+p, col e*128+c = w[e*128+p, t*128+c].

    So a [128, E] slice at row offset t*128 holds, for partition p, the
    concatenation over e of w[e*128+p, t*128:(t+1)*128].
    """
    w4 = w.reshape(E // P, P, H // P, P)          # [e, p, t, c]
    return np.ascontiguousarray(
        w4.transpose(2, 1, 0, 3).reshape(H, E))   # [t, p, e, c] -> [H, E]


def _pack_xt(x):
    """x [S,E] -> xtp [128, NSC*ET*512]: xtp[p, (c*ET+e)*512+s] = x[c*512+s, e*128+p].

    Chunk-major so each 512-wide s-chunk is one contiguous 8KB-per-partition
    DMA into the [P, NSC, ET, 512] SBUF tile.
    """
    x4 = x.reshape(S // 512, 512, E // P, P)          # [c, s, e, p]
    return np.ascontiguousarray(
        x4.transpose(3, 0, 2, 1).reshape(P, -1))      # [p, c, e, s]


def kernel(inputs, Wq, bq, Wk, bk, Wv, bv, _trace=False, _tmpdir=None):
    inputs = np.asarray(inputs, dtype=np.float32)
    wq_f = np.asarray(Wq, dtype=np.float32)
    wk_f = np.asarray(Wk, dtype=np.float32)
    bq_f = np.asarray(bq, dtype=np.float32)
    # softmax-invariant reduction: scores = x (Wq Wk^T) x^T + beta_j + row-const
    m_f = wq_f @ wk_f.T                               # [E, E]
    mtp = _tile_by_h(m_f.astype(NPBF16))
    wtil = wk_f @ bq_f                                # [E]
    inv_sqrt_h = 1.0 / float(np.sqrt(H))
    wv_b = np.ascontiguousarray(np.asarray(Wv, dtype=np.float32).astype(NPBF16))
    bvb = np.ascontiguousarray(
        np.broadcast_to(np.asarray(bv, dtype=np.float32).astype(NPBF16),
                        (P, H)))
    nc = build_program()
    in_maps = []
    for c in range(N_CORES):
        beta = (inputs[c] @ wtil) * inv_sqrt_h        # [S] f32
        bsc = np.ascontiguousarray(beta.reshape(S // P, P).T)  # [128, 16]
        in_maps.append({
            "xt": _pack_xt(inputs[c].astype(NPBF16)),
            "mtp": mtp,
            "wv": wv_b, "bvb": bvb,
            "bsc": bsc.astype(np.float32),
            "vones": np.ones((P, S // P, 2), dtype=NPBF16),
        })
    res = bass_utils.run_bass_kernel_spmd(
        nc, in_maps, core_ids=list(range(N_CORES)),
        trace=_trace, tmpdir=_tmpdir)
    out = np.stack([res.results[c]["out"].astype(np.float32)
                    for c in range(N_CORES)], axis=0)
    if _trace:
        kernel.last_results = res
    return out


# revision 28
# speedup vs baseline: 1.0126x; 1.0126x over previous
"""Single-head causal attention kernel for Trainium2 (Bass/Tile), SPMD over 8 cores.

Problem: inputs [B=8, S=2048, E=1024]; Wq/Wk/Wv [E, H=1024]; bq/bk/bv [H].
  q = x@Wq+bq; k = x@Wk+bk; v = x@Wv+bv
  out = softmax(causal(q k^T / sqrt(H))) v        -> [B, S, H]

Sharding: data-parallel over batch, 1 batch element per NeuronCore (8 cores).

Strategy (v4, bf16 + merged-QK):  softmax is invariant to per-row shifts, so
  q k^T = (x Wq + bq)(x Wk + bk)^T
        = x (Wq Wk^T) x^T + [per-key bias beta_j = x_j.(Wk bq)] + row-consts.
Host precomputes M = Wq Wk^T (f32) and beta (f32), so the device needs only
TWO projections (G^T = M^T x^T and V) instead of three; beta rides for free as
the per-partition bias of the exp() eviction.  x^T stays resident in SBUF and
serves as the stationary side of the scores matmuls (the role K^T played).

  phase 1 (per 512-wide s-chunk): G^T[f,s] (stationary M^T tiles, plain
           eviction), then V[s,h] (stationary x^T tiles; bias added during
           eviction from a host-broadcast [128,H] bv tile). All matmuls N=512.
  phase 2 (q-chunks processed in reverse so the tail chain is the smallest):
           scoresT[k,q] matmuls (stationary x^T k-slices, moving G^T),
           exp(x/32 + beta_k/32) on ScalarE, causal edge mask via
           gpsimd.affine_select; AV + Z share one stationary load per attnT
           tile (fully-masked diagonal tiles skipped for even q-subtiles);
           1/Z folded into the PSUM eviction. AV of a chunk is issued after
           the next chunk's scores so the PE in-order queue never stalls on
           ScalarE.
  A short burst of dummy matmuls runs during the initial DMA wait to lift
  the PE HAM clock-gate to 8/8 before real work starts.
"""

import numpy as np
import ml_dtypes

import concourse.bacc as bacc
import concourse.mybir as mybir
from concourse import tile
from concourse import bass_utils

P = 128
F32 = mybir.dt.float32
BF16 = mybir.dt.bfloat16

B, S, E, H = 8, 2048, 1024, 1024
QC = 256          # q-chunk width in attention phase
N_CORES = 8
NPBF16 = ml_dtypes.bfloat16


def attention_kernel(tc, out, xt, mtp, wv, bvb, bsc, vones):
    nc = tc.nc
    ST, ET, HT = S // P, E // P, H // P     # 128-tiles per dim
    NSC = S // 512                          # 512-wide s-chunks
    NQC = S // QC                           # q-chunks
    QSUB = QC // P
    inv_sqrt_h = 1.0 / float(np.sqrt(H))

    from contextlib import ExitStack

    root = ExitStack()
    with root:
        # ---- constants ----
        const = root.enter_context(tc.tile_pool(name="const", bufs=1))
        warm_src = const.tile([P, 512], BF16, name="warm_src")
        nc.gpsimd.memset(warm_src, 0.0)
        bsc_sb = const.tile([P, ST], F32, name="bsc_sb")
        bv_sb = const.tile([P, H], BF16, name="bv_sb")

        # ---- resident arrays ----
        kqv_pool = root.enter_context(tc.tile_pool(name="kqv", bufs=1))
        # chunk-major: [:, c, e, :] is one contiguous 8KB/partition DMA
        xt_sb = kqv_pool.tile([P, NSC, ET, 512], BF16, name="xt_sb")
        gt = kqv_pool.tile([P, HT, S], BF16, name="gt")      # G^T [f,s]
        # V [s,h] padded with two ones-columns at h=H,H+1 so the softmax
        # row-sum Z rides the AV matmuls (no separate N=1 Z matmuls) and the
        # three AV slices are a uniform 342 wide.
        HP = H + 2
        v_sb = kqv_pool.tile([P, ST, HP], BF16, name="v_sb")

        # ================= phase 1: projections =================
        with ExitStack() as ph1:
            w_pool = ph1.enter_context(tc.tile_pool(name="w", bufs=1))
            # mg_sb[:, t, e, :] = M[e*128+p, t*128+c]  (host pre-tiled)
            mg_sb = w_pool.tile([P, HT, ET, P], BF16, name="mg_sb")
            wv_sb = w_pool.tile([P, ET, H], BF16, name="wv_sb")

            # DMA order. Engines boot staggered: Scalar's queue is live at
            # ~3.5us while Sync (booted last) can't trigger before ~10us, so
            # the critical head of the stream (M tiles 0-2 + xt chunk 0)
            # goes out on the Scalar queue; the rest follows on Sync (each
            # trigger costs ~0.6us of issue time, so few, large transfers).
            CW = ET * 512                        # xt cols per chunk
            nc.scalar.dma_start(mg_sb[:, 0, :, :], mtp[0:P, :])
            nc.scalar.dma_start(mg_sb[:, 1, :, :], mtp[P:2 * P, :])
            nc.scalar.dma_start(mg_sb[:, 2, :, :], mtp[2 * P:3 * P, :])
            nc.sync.dma_start(xt_sb[:, 0, :, :], xt[:, 0:CW])
            for t in range(3, HT):
                nc.sync.dma_start(mg_sb[:, t, :, :], mtp[t * P:(t + 1) * P, :])
            for c in range(1, NSC):
                nc.sync.dma_start(xt_sb[:, c, :, :],
                                  xt[:, c * CW:(c + 1) * CW])
            for e in range(ET):
                nc.sync.dma_start(wv_sb[:, e, :], wv[e * P:(e + 1) * P, :])
            nc.sync.dma_start(bv_sb[:], bvb)
            nc.sync.dma_start(bsc_sb[:], bsc)
            nc.sync.dma_start(v_sb[:, :, H:HP], vones)

            # gpsum first: phase-2 pools then reuse banks that phase 1
            # frees early (after G) rather than last (after V's evictions).
            gpsum = ph1.enter_context(tc.tile_pool(name="gpsum", bufs=2,
                                                   space="PSUM"))
            vpsum = ph1.enter_context(tc.tile_pool(name="vpsum", bufs=2,
                                                   space="PSUM"))
            # HAM warmup: dummy matmuls with no DMA dependency fill the
            # initial DMA wait and lift the PE clock gate to 8/8.
            wp = gpsum.tile([P, 512], F32, name="g0", space="PSUM")
            for _ in range(10):
                nc.tensor.matmul(wp[:], warm_src[:, 0:P], warm_src[:],
                                 start=True, stop=True)

            def evict_g(t, c, psum, alt):
                if alt % 2 == 0:
                    nc.scalar.copy(gt[:, t, c * 512:(c + 1) * 512], psum[:])
                else:
                    nc.vector.tensor_copy(gt[:, t, c * 512:(c + 1) * 512],
                                          psum[:])

            def v_chunk(c):
                for si in range(4 * c, 4 * c + 4):
                    vps = []
                    for hc in range(2):
                        vp = vpsum.tile([P, 512], F32, name="vp", space="PSUM")
                        for e in range(ET):
                            nc.tensor.matmul(
                                vp[:],
                                xt_sb[:, c, e, (si % 4) * P:(si % 4 + 1) * P],
                                wv_sb[:, e, hc * 512:(hc + 1) * 512],
                                start=(e == 0), stop=(e == ET - 1))
                        vps.append(vp)
                    for hc in range(2):
                        nc.vector.scalar_tensor_tensor(
                            v_sb[:, si, hc * 512:(hc + 1) * 512], vps[hc][:],
                            1.0, bv_sb[:, hc * 512:(hc + 1) * 512],
                            mybir.AluOpType.mult, mybir.AluOpType.add)

            # ---- G: plain per-chunk loops; chunk-local deps keep the PE
            # fed as each xt chunk lands (chunk c is consumed over ~13.6us
            # while chunk c+1 streams in).
            for c in range(NSC):
                for t in range(HT):
                    gp = gpsum.tile([P, 512], F32, name=f"g{c % 2}",
                                    space="PSUM")
                    for e in range(ET):
                        nc.tensor.matmul(
                            gp[:], mg_sb[:, t, e, :], xt_sb[:, c, e, :],
                            start=(e == 0), stop=(e == ET - 1))
                    evict_g(t, c, gp, t + c)
            for c in range(NSC):
                v_chunk(c)

        # ================= phase 2: attention =================
        with ExitStack() as ph2:
            attn_pool = ph2.enter_context(
                tc.tile_pool(name="attnT", bufs=36))
            o_pool = ph2.enter_context(tc.tile_pool(name="o_stage", bufs=3))
            rz_pool = ph2.enter_context(tc.tile_pool(name="rz", bufs=4))
            spsum = ph2.enter_context(tc.tile_pool(name="spsum", bufs=2,
                                                   space="PSUM"))
            opsum = ph2.enter_context(tc.tile_pool(name="opsum", bufs=6,
                                                   space="PSUM"))

            def scores_chunk(j):
                """ScoresT tiles [k,q] + exp + causal mask for q-chunk j."""
                nk = ((j + 1) * QC) // P
                ats = []
                for i in range(nk):
                    # The last k-tile (i == 2j+1) lies above the diagonal for
                    # the first q-subtile; only its [:, P:2P] half is ever
                    # read by AV/Z, so compute just those 128 columns.
                    lo = P if i == nk - 1 else 0
                    sp = spsum.tile([P, QC], F32, name="sp", space="PSUM")
                    for t in range(HT):
                        nc.tensor.matmul(
                            sp[:, 0:QC - lo],
                            xt_sb[:, i // 4, t,
                                  (i % 4) * P:(i % 4 + 1) * P],
                            gt[:, t, j * QC + lo:(j + 1) * QC],
                            start=(t == 0), stop=(t == HT - 1))
                    at = attn_pool.tile([P, QC], BF16, name="at")
                    nc.scalar.activation(at[:, lo:QC], sp[:, 0:QC - lo],
                                         mybir.ActivationFunctionType.Exp,
                                         scale=inv_sqrt_h,
                                         bias=bsc_sb[:, i:i + 1])
                    if (i + 1) * P > j * QC + lo:   # tile touches the diagonal
                        nc.gpsimd.affine_select(
                            out=at[:, lo:QC], in_=at[:, lo:QC],
                            compare_op=mybir.AluOpType.is_ge,
                            fill=0.0,
                            base=j * QC + lo - i * P,
                            channel_multiplier=-1,
                            pattern=[[1, QC - lo]])
                    ats.append(at)
                return ats

            # AV splits H+2 (V plus the ones columns) into 3 PSUM-bank-sized
            # matmuls; Z comes out as the last columns of the third slice.
            AVS = [(0, 342), (342, 342), (684, 342)]

            def av_chunk(j, ats):
                """AV + Z for q-chunk j given its masked attnT tiles."""
                for qs in range(QSUB - 1, -1, -1):   # larger qs first
                    # causal: k-tiles above the diagonal for this q-subtile
                    # are fully masked; skip them.
                    nk = 2 * j + qs + 1
                    ops = [opsum.tile([P, 342], F32, name="op", space="PSUM")
                           for _ in range(3)]
                    for i in range(nk):
                        a_sl = ats[i][:, qs * P:(qs + 1) * P]
                        for sl, (off, w) in enumerate(AVS):
                            nc.tensor.matmul(
                                ops[sl][:, 0:w], a_sl,
                                v_sb[:, i, off:off + w],
                                start=(i == 0), stop=(i == nk - 1))
                    rz = rz_pool.tile([P, 1], F32, name="rz")
                    nc.vector.reciprocal(rz[:], ops[2][:, 340:341])
                    o_st = o_pool.tile([P, H], BF16, name="o_st")
                    row = j * QC + qs * P
                    # eviction muls split across Vector and Scalar so the
                    # per-subtile serial chain stays under the AV matmul time
                    nc.vector.tensor_scalar_mul(
                        o_st[:, 0:342], ops[0][:, 0:342], rz[:, 0:1])
                    nc.scalar.mul(o_st[:, 342:684], ops[1][:, 0:342],
                                  rz[:, 0:1])
                    if qs % 2 == 0:
                        nc.vector.tensor_scalar_mul(
                            o_st[:, 684:1024], ops[2][:, 0:340], rz[:, 0:1])
                    else:
                        nc.scalar.mul(o_st[:, 684:1024], ops[2][:, 0:340],
                                      rz[:, 0:1])
                    nc.sync.dma_start(out[row:row + P, :], o_st[:])

            prev = None
            prev_j = None
            for j in range(NQC - 1, -1, -1):     # reverse: smallest AV last
                ats = scores_chunk(j)
                if prev is not None:
                    av_chunk(prev_j, prev)
                prev, prev_j = ats, j
            av_chunk(prev_j, prev)


def build_program(n_cores=N_CORES):
    nc = bacc.Bacc("TRN2", target_bir_lowering=False, debug=False,
                   num_devices=n_cores)
    xt = nc.dram_tensor("xt", [P, S * E // P], BF16, kind="ExternalInput").ap()
    mtp = nc.dram_tensor("mtp", [H, E], BF16, kind="ExternalInput").ap()
    wv = nc.dram_tensor("wv", [E, H], BF16, kind="ExternalInput").ap()
    bvb = nc.dram_tensor("bvb", [P, H], BF16, kind="ExternalInput").ap()
    bsc = nc.dram_tensor("bsc", [P, S // P], F32, kind="ExternalInput").ap()
    vones = nc.dram_tensor("vones", [P, S // P, 2], BF16,
                           kind="ExternalInput").ap()
    out = nc.dram_tensor("out", [S, H], BF16, kind="ExternalOutput").ap()
    with tile.TileContext(nc) as tc:
        attention_kernel(tc, out, xt, mtp, wv, bvb, bsc, vones)
    nc.compile()
    return nc


def _tile_by_h(w):
    """[E,H] -> [H,E] layout where row t*128+p, col e*128+c = w[e*128+p, t*128+c].

    So a [128, E] slice at row offset t*128 holds, for partition p, the
    concatenation over e of w[e*128+p, t*128:(t+1)*128].
    """
    w4 = w.reshape(E // P, P, H // P, P)          # [e, p, t, c]
    return np.ascontiguousarray(
        w4.transpose(2, 1, 0, 3).reshape(H, E))   # [t, p, e, c] -> [H, E]


def _pack_xt(x):
    """x [S,E] -> xtp [128, NSC*ET*512]: xtp[p, (c*ET+e)*512+s] = x[c*512+s, e*128+p].

    Chunk-major so each 512-wide s-chunk is one contiguous 8KB-per-partition
    DMA into the [P, NSC, ET, 512] SBUF tile.
    """
    x4 = x.reshape(S // 512, 512, E // P, P)          # [c, s, e, p]
    return np.ascontiguousarray(
        x4.transpose(3, 0, 2, 1).reshape(P, -1))      # [p, c, e, s]


def kernel(inputs, Wq, bq, Wk, bk, Wv, bv, _trace=False, _tmpdir=None):
    inputs = np.asarray(inputs, dtype=np.float32)
    wq_f = np.asarray(Wq, dtype=np.float32)
    wk_f = np.asarray(Wk, dtype=np.float32)
    bq_f = np.asarray(bq, dtype=np.float32)
    # softmax-invariant reduction: scores = x (Wq Wk^T) x^T + beta_j + row-const
    m_f = wq_f @ wk_f.T                               # [E, E]
    mtp = _tile_by_h(m_f.astype(NPBF16))
    wtil = wk_f @ bq_f                                # [E]
    inv_sqrt_h = 1.0 / float(np.sqrt(H))
    wv_b = np.ascontiguousarray(np.asarray(Wv, dtype=np.float32).astype(NPBF16))
    bvb = np.ascontiguousarray(
        np.broadcast_to(np.asarray(bv, dtype=np.float32).astype(NPBF16),
                        (P, H)))
    nc = build_program()
    in_maps = []
    for c in range(N_CORES):
        beta = (inputs[c] @ wtil) * inv_sqrt_h        # [S] f32
        bsc = np.ascontiguousarray(beta.reshape(S // P, P).T)  # [128, 16]
        in_maps.append({
            "xt": _pack_xt(inputs[c].astype(NPBF16)),
            "mtp": mtp,
            "wv": wv_b, "bvb": bvb,
            "bsc": bsc.astype(np.float32),
            "vones": np.ones((P, S // P, 2), dtype=NPBF16),
        })
    res = bass_utils.run_bass_kernel_spmd(
        nc, in_maps, core_ids=list(range(N_CORES)),
        trace=_trace, tmpdir=_tmpdir)
    out = np.stack([res.results[c]["out"].astype(np.float32)
                    for c in range(N_CORES)], axis=0)
    if _trace:
        kernel.last_results = res
    return out
